# revision 5
# baseline (speedup 1.0000x reference)
"""Trainium2 Bass kernel for nn_Attention (B=8, L=2048, D=64).

Reference (per batch b):
    BZ = x @ B_w.T + B_b
    CZ = x @ C_w.T + C_b
    scores = BZ @ CZ.T              # (L, L)
    attn = relu(scores)
    attn = attn / (attn.sum(axis=-2, keepdims=True) + EPS)   # column-sum norm
    VZ = x @ V_w.T + V_b
    out = x + attn @ VZ

Strategy (one batch per NeuronCore, 8 cores, no cross-core comms):
  * Host pre-transposes x into x_aug^T = [x.T; 1] (65 x 2048, fp16) so the
    kernel never runs PE transposes; biases fold via the augmented row.
  * Projections BZ^T / CZ^T computed into BOTH partition halves directly
    (PE column tiling h0/h64) so the K=64 scores matmuls can be
    row-packed: two m-chunks run concurrently in PE row groups.
  * S^T orientation puts the column-normalization axis on the free dim;
    relu + column-sums fuse into the PSUM->SBUF evacuation
    (ACT activation accum_out / DVE tensor_scalar accum_out, one engine
    per 1024-wide half).  This evacuation is the hard throughput floor
    (ACT+DVE are the only engines with PSUM ports).
  * den merge(+eps) and the VZ row scaling run on GpSimd (SBUF-only ops)
    so the two PSUM-capable engines do nothing but evacuate.
  * Normalization folds into VZ rows: O^T = (VZ*recip)^T @ A^T,
    accumulated in PSUM over m-chunks, column-packed into [128, 1024]
    (l lower half on partitions 0-63, upper on 64-127).
  * The +x residual is 4 identity matmuls accumulated into the same
    PSUM ahead of the O matmuls (start=True), so the epilogue is just
    PSUM evac + DMA out; the host un-packs/transposes the [128,1024]
    result back to (2048, 64).
  * Software-pipelined emission: O matmuls trail the scores of the next
    chunk pair so the in-order PE never stalls on the relu/normalize
    chain.
  * All PE matmuls run in fp16 (fp32 PSUM accumulation).
"""

import os
import sys

sys.path.insert(0, "/opt/trn_rl_repo")

import numpy as np

import concourse.bacc as bacc
import concourse.tile as tile
from concourse import mybir
from concourse import bass_utils

L = 2048
D = 64
DA = D + 1          # augmented feature dim
P = 128
NCH = L // P        # 16 m-chunks
SL = 512            # matmul moving-slice width
NSL = L // SL       # 4 slices
EU = 1024           # relu-evacuation unit width (2 PSUM banks)
EPS = 1e-8
N_CORES = 8

F32 = mybir.dt.float32
F16 = mybir.dt.float16


def _attention_kernel(ctx, tc, yt_ap, xat_ap, b_ap, c_ap, v_ap, i_ap, cfg):
    nc = tc.nc
    Relu = mybir.ActivationFunctionType.Relu
    Copy = mybir.ActivationFunctionType.Copy
    Alu = mybir.AluOpType
    at_dt = F16

    consts = ctx.enter_context(tc.tile_pool(name="consts", bufs=1))
    bigs = ctx.enter_context(tc.tile_pool(name="bigs", bufs=1))
    at_pool = ctx.enter_context(tc.tile_pool(name="at", bufs=5))
    small = ctx.enter_context(tc.tile_pool(name="small", bufs=8))

    b_sb = consts.tile([DA, D], F16)
    nc.sync.dma_start(out=b_sb, in_=b_ap)
    c_sb = consts.tile([DA, D], F16)
    nc.sync.dma_start(out=c_sb, in_=c_ap)
    v_sb = consts.tile([DA, D], F16)
    nc.sync.dma_start(out=v_sb, in_=v_ap)
    ident = consts.tile([D, D], F16)
    nc.sync.dma_start(out=ident, in_=i_ap)

    # x_aug^T, host-prepared: 4 column slices on 2 HWDGE queues
    xT = bigs.tile([DA, L], F16)
    for j in range(NSL):
        eng = nc.sync if j % 2 == 0 else nc.scalar
        eng.dma_start(out=xT[:, SL * j : SL * (j + 1)],
                      in_=xat_ap[:, SL * j : SL * (j + 1)])

    bz = bigs.tile([P, L], F16)           # BZ^T duplicated on both halves
    cz = bigs.tile([P, L], F16)           # CZ^T duplicated on both halves
    vz_sb = bigs.tile([P, NCH, D], F32)   # VZ natural
    yt_sb = bigs.tile([P, EU], F32)       # O^T + x^T col-packed staging

    # ---------------- prologue ----------------
    # Projections run first and double as the HAM warmup burst.
    with tc.tile_pool(name="pp", bufs=3, space="PSUM") as pp_pool, \
         tc.tile_pool(name="pv", bufs=2, space="PSUM") as pv_pool:
        # BZ^T / CZ^T: each [128, 2048] via two [128, 1024] PSUM tiles;
        # h0/h64 column-group pairs run concurrently on the PE.
        for w_sb, dst in ((b_sb, bz), (c_sb, cz)):
            for u in range(2):
                pp = pp_pool.tile([P, EU], F32, tag="pp")
                for jj in range(2):
                    j = 2 * u + jj
                    sl = slice(SL * jj, SL * (jj + 1))
                    xs = xT[:, SL * j : SL * (j + 1)]
                    nc.tensor.matmul(pp[0:D, sl], w_sb, xs,
                                     start=True, stop=True)
                    nc.tensor.matmul(pp[D : 2 * D, sl], w_sb, xs,
                                     start=True, stop=True)
                if u == 0:
                    nc.scalar.activation(out=dst[:, 0:EU], in_=pp, func=Copy)
                else:
                    nc.vector.tensor_copy(dst[:, EU : 2 * EU], pp)
        # VZ natural: 8 chunks per PSUM bank
        for g in range(2):
            pv = pv_pool.tile([P, 8, D], F32)
            for j in range(8):
                c = 8 * g + j
                nc.tensor.matmul(pv[:, j, :],
                                 xT[:, P * c : P * (c + 1)],
                                 v_sb, start=True, stop=True)
            nc.scalar.activation(out=vz_sb[:, 8 * g : 8 * (g + 1), :],
                                 in_=pv, func=Copy)

    # ---------------- main loop ----------------
    # Chunk PAIR (cA rows 0-63, cB rows 64-127, row-packed so both score
    # matmuls run concurrently in separate PE row groups), relu+colsum
    # evacuation, normalization folded into VZ; O^T accumulation emitted
    # one pair behind so the in-order PE never waits on the chain.
    def emit_chain(cA, cB, cs2A, cs2B):
        # den = cs2[0] + cs2[1] per chunk (EPS folded into the DVE
        # evacuation op); off the PSUM-capable engines
        csAB = small.tile([P, 2], F32, tag="csAB")
        meng = nc.gpsimd if cfg["gps_merge"] else nc.vector
        meng.tensor_tensor(out=csAB[:, 0:1], in0=cs2A[:, 0:1],
                           in1=cs2A[:, 1:2], op=Alu.add)
        meng.tensor_tensor(out=csAB[:, 1:2], in0=cs2B[:, 0:1],
                           in1=cs2B[:, 1:2], op=Alu.add)
        recip = small.tile([P, 2], F32, tag="recip")
        nc.vector.reciprocal(recip, csAB)
        vzsA = small.tile([P, D], at_dt, tag="vzsA")
        vzsB = small.tile([P, D], at_dt, tag="vzsB")
        if cfg["gps_scale"]:
            nc.gpsimd.tensor_scalar(out=vzsA, in0=vz_sb[:, cA, :],
                                    scalar1=recip[:, 0:1], scalar2=None,
                                    op0=Alu.mult)
            nc.gpsimd.tensor_scalar(out=vzsB, in0=vz_sb[:, cB, :],
                                    scalar1=recip[:, 1:2], scalar2=None,
                                    op0=Alu.mult)
        else:
            nc.scalar.activation(out=vzsA, in_=vz_sb[:, cA, :], func=Copy,
                                 scale=recip[:, 0:1])
            nc.scalar.activation(out=vzsB, in_=vz_sb[:, cB, :], func=Copy,
                                 scale=recip[:, 1:2])
        return vzsA, vzsB

    def emit_o(c, at, vzs):
        # column-packed: j 0/1 -> partitions 0-63, j 2/3 -> 64-127;
        # interleave col groups so both halves overlap on the PE
        for j in (0, 2, 1, 3):
            if j < 2:
                out_ap = po[0:D, SL * j : SL * (j + 1)]
            else:
                out_ap = po[D : 2 * D, SL * (j - 2) : SL * (j - 1)]
            nc.tensor.matmul(out_ap, vzs, at[:, SL * j : SL * (j + 1)],
                             start=False, stop=(c == NCH - 1))

    with tc.tile_pool(name="po", bufs=1, space="PSUM") as po_pool:
        po = po_pool.tile([P, EU], F32)    # O^T col-packed (2 banks)
        # +x residual: identity matmuls seed the O^T accumulator
        for j in range(4):
            if j < 2:
                out_ap = po[0:D, SL * j : SL * (j + 1)]
            else:
                out_ap = po[D : 2 * D, SL * (j - 2) : SL * (j - 1)]
            nc.tensor.matmul(out_ap, ident, xT[0:D, SL * j : SL * (j + 1)],
                             start=True, stop=False)
        with tc.tile_pool(name="ps", bufs=3, space="PSUM") as ps_pool:
            prev = None
            for p in range(NCH // 2):
                cA, cB = 2 * p, 2 * p + 1
                atA = at_pool.tile([P, L], at_dt, tag="at")
                atB = at_pool.tile([P, L], at_dt, tag="at")
                cs2A = small.tile([P, 2], F32, tag="cs2A")
                cs2B = small.tile([P, 2], F32, tag="cs2B")
                pstiles = {}
                for u in range(2):
                    psA = ps_pool.tile([P, EU], F32, tag="ps")
                    psB = ps_pool.tile([P, EU], F32, tag="ps")
                    pstiles[("A", u)] = psA
                    pstiles[("B", u)] = psB
                    for jj in range(2):
                        j = 2 * u + jj
                        nc.tensor.matmul(psA[:, SL * jj : SL * (jj + 1)],
                                         cz[0:D, P * cA : P * (cA + 1)],
                                         bz[0:D, SL * j : SL * (j + 1)],
                                         start=True, stop=True)
                        nc.tensor.matmul(psB[:, SL * jj : SL * (jj + 1)],
                                         cz[D : 2 * D, P * cB : P * (cB + 1)],
                                         bz[D : 2 * D, SL * j : SL * (j + 1)],
                                         start=True, stop=True)
                # relu + column-sum evacuation: ACT unit u=0, DVE unit
                # u=1 for each chunk
                for nm, at, cs2 in (("A", atA, cs2A), ("B", atB, cs2B)):
                    nc.scalar.activation(
                        out=at[:, 0:EU], in_=pstiles[(nm, 0)],
                        func=Relu, accum_out=cs2[:, 0:1])
                    nc.vector.tensor_scalar(
                        out=at[:, EU : 2 * EU], in0=pstiles[(nm, 1)],
                        scalar1=0.0, scalar2=EPS,
                        op0=Alu.max, op1=Alu.add,
                        accum_out=cs2[:, 1:2])
                vzsA, vzsB = emit_chain(cA, cB, cs2A, cs2B)
                if prev is not None:
                    emit_o(prev[0], prev[1], prev[2])
                    emit_o(prev[3], prev[4], prev[5])
                prev = (cA, atA, vzsA, cB, atB, vzsB)
            emit_o(prev[0], prev[1], prev[2])
            emit_o(prev[3], prev[4], prev[5])

        # ---------------- epilogue ----------------
        nc.scalar.activation(out=yt_sb[:, 0 : EU // 2],
                             in_=po[:, 0 : EU // 2], func=Copy)
        nc.vector.tensor_copy(yt_sb[:, EU // 2 : EU], po[:, EU // 2 : EU])
        for j in range(4):
            eng = nc.sync if j % 2 == 0 else nc.scalar
            eng.dma_start(out=yt_ap[:, 256 * j : 256 * (j + 1)],
                          in_=yt_sb[:, 256 * j : 256 * (j + 1)])


_CACHE = {}


def _build(gps_merge=True, gps_scale=True):
    key = ("nc", gps_merge, gps_scale)
    if key in _CACHE:
        return _CACHE[key]
    cfg = {"gps_merge": gps_merge, "gps_scale": gps_scale}
    nc = bacc.Bacc("TRN2", target_bir_lowering=False, debug=False,
                   enable_asserts=False, num_devices=1)
    xat = nc.dram_tensor("xat", (DA, L), F16, kind="ExternalInput").ap()
    b = nc.dram_tensor("b_augt", (DA, D), F16, kind="ExternalInput").ap()
    c = nc.dram_tensor("c_augt", (DA, D), F16, kind="ExternalInput").ap()
    v = nc.dram_tensor("v_augt", (DA, D), F16, kind="ExternalInput").ap()
    i = nc.dram_tensor("ident", (D, D), F16, kind="ExternalInput").ap()
    yt = nc.dram_tensor("yt", (P, EU), F32, kind="ExternalOutput").ap()
    from contextlib import ExitStack
    with tile.TileContext(nc) as tc, ExitStack() as ctx:
        _attention_kernel(ctx, tc, yt, xat, b, c, v, i, cfg)
    nc.compile()
    _CACHE[key] = nc
    return nc


def _fold_weights(B_w, B_b, C_w, C_b, V_w, V_b):
    def aug(w, bias):
        full = np.concatenate(
            [np.asarray(w, np.float32).T, np.asarray(bias, np.float32)[None, :]],
            axis=0)
        return full.astype(np.float16)
    return aug(B_w, B_b), aug(C_w, C_b), aug(V_w, V_b)


def run(inputs, trace=False, tmpdir=None, gps_merge=True, gps_scale=True):
    nc = _build(gps_merge, gps_scale)
    x = np.asarray(inputs["x"], dtype=np.float32)
    b_augt, c_augt, v_augt = _fold_weights(
        inputs["B_w"], inputs["B_b"], inputs["C_w"], inputs["C_b"],
        inputs["V_w"], inputs["V_b"])
    ident = np.eye(D, dtype=np.float16)
    ones = np.ones((1, L), np.float16)
    in_maps = []
    for i in range(N_CORES):
        xat = np.concatenate(
            [np.ascontiguousarray(x[i].T).astype(np.float16), ones], axis=0)
        in_maps.append({"xat": xat, "b_augt": b_augt, "c_augt": c_augt,
                        "v_augt": v_augt, "ident": ident})
    res = bass_utils.run_bass_kernel_spmd(nc, in_maps,
                                          core_ids=list(range(N_CORES)),
                                          trace=trace, tmpdir=tmpdir)
    out = np.empty((N_CORES, L, D), np.float32)
    for i in range(N_CORES):
        yt = res.results[i]["yt"]
        out[i, 0:EU, :] = yt[0:D, :].T
        out[i, EU : 2 * EU, :] = yt[D : 2 * D, :].T
    return out, res


def kernel(**inputs) -> np.ndarray:
    out, _ = run(inputs, trace=False)
    return out


# revision 13
# speedup vs baseline: 1.1928x; 1.1928x over previous
"""Trainium2 Bass kernel for nn_Attention (B=8, L=2048, D=64).

Reference (per batch b):
    BZ = x @ B_w.T + B_b
    CZ = x @ C_w.T + C_b
    scores = BZ @ CZ.T              # (L, L)
    attn = relu(scores)
    attn = attn / (attn.sum(axis=-2, keepdims=True) + EPS)   # column-sum norm
    VZ = x @ V_w.T + V_b
    out = x + attn @ VZ

Strategy (one batch per NeuronCore, 8 cores, no cross-core comms):
  * Host pre-transposes x into x_aug^T = [x.T; 1] (65 x 2048, fp16) so the
    kernel never runs PE transposes; biases fold via the augmented row.
  * Projections BZ^T / CZ^T computed into BOTH partition halves directly
    (PE column tiling h0/h64) so the K=64 scores matmuls can be
    row-packed: two m-chunks run concurrently in PE row groups.
  * S^T orientation puts the column-normalization axis on the free dim;
    relu + column-sums fuse into the PSUM->SBUF evacuation
    (ACT activation accum_out / DVE tensor_scalar accum_out, one engine
    per 1024-wide half).  This evacuation is the hard throughput floor
    (ACT+DVE are the only engines with PSUM ports).
  * den merge(+eps) and the VZ row scaling run on GpSimd (SBUF-only ops)
    so the two PSUM-capable engines do nothing but evacuate.
  * Normalization folds into VZ rows: O^T = (VZ*recip)^T @ A^T,
    accumulated in PSUM over m-chunks, column-packed into [128, 1024]
    (l lower half on partitions 0-63, upper on 64-127).
  * The +x residual is 4 identity matmuls accumulated into the same
    PSUM ahead of the O matmuls (start=True), so the epilogue is just
    PSUM evac + DMA out; the host un-packs/transposes the [128,1024]
    result back to (2048, 64).
  * Software-pipelined emission: O matmuls trail the scores of the next
    chunk pair so the in-order PE never stalls on the relu/normalize
    chain.
  * All PE matmuls run in fp16 (fp32 PSUM accumulation).
"""

import os
import sys

sys.path.insert(0, "/opt/trn_rl_repo")

import numpy as np

import concourse.bacc as bacc
import concourse.tile as tile
from concourse import mybir
from concourse import bass_utils

L = 2048
D = 64
DA = D + 1          # augmented feature dim
P = 128
NCH = L // P        # 16 m-chunks
SL = 512            # matmul moving-slice width
NSL = L // SL       # 4 slices
EU = 1024           # relu-evacuation unit width (2 PSUM banks)
EPS = 1e-8
N_CORES = 8

F32 = mybir.dt.float32
F16 = mybir.dt.float16


def _attention_kernel(ctx, tc, yt_ap, xat_ap, w_ap, cfg):
    nc = tc.nc
    Relu = mybir.ActivationFunctionType.Relu
    Copy = mybir.ActivationFunctionType.Copy
    Alu = mybir.AluOpType
    at_dt = F16

    consts = ctx.enter_context(tc.tile_pool(name="consts", bufs=1))
    bigs = ctx.enter_context(tc.tile_pool(name="bigs", bufs=1))
    at_pool = ctx.enter_context(tc.tile_pool(name="at", bufs=5))
    small = ctx.enter_context(tc.tile_pool(name="small", bufs=8))

    # one packed weight DMA: [B_aug | C_aug | V_aug | ident]
    w_sb = consts.tile([DA, 4 * D], F16)
    nc.scalar.dma_start(out=w_sb, in_=w_ap)
    b_sb = w_sb[:, 0:D]
    c_sb = w_sb[:, D : 2 * D]
    v_sb = w_sb[:, 2 * D : 3 * D]
    ident = w_sb[0:D, 3 * D : 4 * D]

    # x_aug^T, host-prepared: 4 column slices spread over 3 DMA queues
    xT = bigs.tile([DA, L], F16)
    for j, eng in enumerate((nc.sync, nc.gpsimd, nc.sync, nc.scalar)):
        eng.dma_start(out=xT[:, SL * j : SL * (j + 1)],
                      in_=xat_ap[:, SL * j : SL * (j + 1)])

    # PE warmup burst overlapping the input DMAs: sustained matmul
    # activity trips the HAM clock gate (1.2 -> 2.4 GHz) before real work
    wu_a = consts.tile([P, SL], F16)
    nc.vector.memset(wu_a, 0.25)
    wu_res = consts.tile([P, 1], F32)
    with tc.tile_pool(name="pw", bufs=2, space="PSUM") as pw_pool:
        pw = None
        for i in range(cfg["warmup"]):
            pw = pw_pool.tile([P, SL], F32, tag="pw")
            nc.tensor.matmul(pw, wu_a[:, 0:P], wu_a, start=True, stop=True)
        if pw is not None:
            nc.vector.tensor_copy(wu_res, pw[:, 0:1])

    bz = bigs.tile([P, L], F16)           # BZ^T duplicated on both halves
    cz = bigs.tile([P, L], F16)           # CZ^T duplicated on both halves
    vz_sb = bigs.tile([P, NCH, D], F32)   # VZ natural
    yt_sb = bigs.tile([P, EU], F32)       # O^T + x^T col-packed staging

    # ---------------- prologue ----------------
    # Projections run first and double as the HAM warmup burst.
    with tc.tile_pool(name="pp", bufs=3, space="PSUM") as pp_pool, \
         tc.tile_pool(name="pv", bufs=2, space="PSUM") as pv_pool:
        # BZ^T / CZ^T: each [128, 2048] via two [128, 1024] PSUM tiles;
        # h0/h64 column-group pairs run concurrently on the PE.
        for w_sb, dst in ((b_sb, bz), (c_sb, cz)):
            for u in range(2):
                pp = pp_pool.tile([P, EU], F32, tag="pp")
                for jj in range(2):
                    j = 2 * u + jj
                    sl = slice(SL * jj, SL * (jj + 1))
                    xs = xT[:, SL * j : SL * (j + 1)]
                    nc.tensor.matmul(pp[0:D, sl], w_sb, xs,
                                     start=True, stop=True)
                    nc.tensor.matmul(pp[D : 2 * D, sl], w_sb, xs,
                                     start=True, stop=True)
                if u == 0:
                    nc.scalar.activation(out=dst[:, 0:EU], in_=pp, func=Copy)
                else:
                    nc.vector.tensor_copy(dst[:, EU : 2 * EU], pp)
        # VZ natural: 8 chunks per PSUM bank
        for g in range(2):
            pv = pv_pool.tile([P, 8, D], F32)
            for j in range(8):
                c = 8 * g + j
                nc.tensor.matmul(pv[:, j, :],
                                 xT[:, P * c : P * (c + 1)],
                                 v_sb, start=True, stop=True)
            nc.scalar.activation(out=vz_sb[:, 8 * g : 8 * (g + 1), :],
                                 in_=pv, func=Copy)

    # ---------------- main loop ----------------
    # Chunk PAIR (cA rows 0-63, cB rows 64-127, row-packed so both score
    # matmuls run concurrently in separate PE row groups), relu+colsum
    # evacuation, normalization folded into VZ; O^T accumulation emitted
    # one pair behind so the in-order PE never waits on the chain.
    def emit_chain(cA, cB, cs2A, cs2B):
        # den = cs2[0] + cs2[1] per chunk (EPS folded into the DVE
        # evacuation op); off the PSUM-capable engines
        csAB = small.tile([P, 2], F32, tag="csAB")
        meng = nc.gpsimd if cfg["gps_merge"] else nc.vector
        meng.tensor_tensor(out=csAB[:, 0:1], in0=cs2A[:, 0:1],
                           in1=cs2A[:, 1:2], op=Alu.add)
        meng.tensor_tensor(out=csAB[:, 1:2], in0=cs2B[:, 0:1],
                           in1=cs2B[:, 1:2], op=Alu.add)
        recip = small.tile([P, 2], F32, tag="recip")
        nc.vector.reciprocal(recip, csAB)
        vzsA = small.tile([P, D], at_dt, tag="vzsA")
        vzsB = small.tile([P, D], at_dt, tag="vzsB")
        # SBUF-only fp32 tensor_scalar: DVE runs it in 2x mode, cheap
        nc.vector.tensor_scalar(out=vzsA, in0=vz_sb[:, cA, :],
                                scalar1=recip[:, 0:1], scalar2=None,
                                op0=Alu.mult)
        nc.vector.tensor_scalar(out=vzsB, in0=vz_sb[:, cB, :],
                                scalar1=recip[:, 1:2], scalar2=None,
                                op0=Alu.mult)
        return vzsA, vzsB

    def emit_o(c, at, vzs):
        # column-packed: j 0/1 -> partitions 0-63, j 2/3 -> 64-127.
        # Deliberately NOT interleaved for col-group overlap: the extra
        # serialized PE time keeps the PE dense (evacuation engines set
        # the pace), which holds the HAM clock gate at 2.4 GHz.
        for j in (0, 1, 2, 3):
            if j < 2:
                out_ap = po[0:D, SL * j : SL * (j + 1)]
            else:
                out_ap = po[D : 2 * D, SL * (j - 2) : SL * (j - 1)]
            nc.tensor.matmul(out_ap, vzs, at[:, SL * j : SL * (j + 1)],
                             start=False, stop=(c == NCH - 1))

    with tc.tile_pool(name="po", bufs=1, space="PSUM") as po_pool:
        po = po_pool.tile([P, EU], F32)    # O^T col-packed (2 banks)
        # +x residual: identity matmuls seed the O^T accumulator
        for j in range(4):
            if j < 2:
                out_ap = po[0:D, SL * j : SL * (j + 1)]
            else:
                out_ap = po[D : 2 * D, SL * (j - 2) : SL * (j - 1)]
            nc.tensor.matmul(out_ap, ident, xT[0:D, SL * j : SL * (j + 1)],
                             start=True, stop=False)
        with tc.tile_pool(name="ps", bufs=3, space="PSUM") as ps_pool:
            prev = None
            for p in range(NCH // 2):
                cA, cB = 2 * p, 2 * p + 1
                atA = at_pool.tile([P, L], at_dt, tag="at")
                atB = at_pool.tile([P, L], at_dt, tag="at")
                cs2A = small.tile([P, 2], F32, tag="cs2A")
                cs2B = small.tile([P, 2], F32, tag="cs2B")
                pstiles = {}
                for u in range(2):
                    psA = ps_pool.tile([P, EU], F32, tag="ps")
                    psB = ps_pool.tile([P, EU], F32, tag="ps")
                    pstiles[("A", u)] = psA
                    pstiles[("B", u)] = psB
                    for jj in range(2):
                        j = 2 * u + jj
                        nc.tensor.matmul(psA[:, SL * jj : SL * (jj + 1)],
                                         cz[0:D, P * cA : P * (cA + 1)],
                                         bz[0:D, SL * j : SL * (j + 1)],
                                         start=True, stop=True)
                        nc.tensor.matmul(psB[:, SL * jj : SL * (jj + 1)],
                                         cz[D : 2 * D, P * cB : P * (cB + 1)],
                                         bz[D : 2 * D, SL * j : SL * (j + 1)],
                                         start=True, stop=True)
                # relu + column-sum evacuation: ACT unit u=0, DVE unit
                # u=1 for each chunk
                for nm, at, cs2 in (("A", atA, cs2A), ("B", atB, cs2B)):
                    nc.scalar.activation(
                        out=at[:, 0:EU], in_=pstiles[(nm, 0)],
                        func=Relu, accum_out=cs2[:, 0:1])
                    nc.vector.tensor_scalar(
                        out=at[:, EU : 2 * EU], in0=pstiles[(nm, 1)],
                        scalar1=0.0, scalar2=EPS,
                        op0=Alu.max, op1=Alu.add,
                        accum_out=cs2[:, 1:2])
                vzsA, vzsB = emit_chain(cA, cB, cs2A, cs2B)
                if prev is not None:
                    emit_o(prev[0], prev[1], prev[2])
                    emit_o(prev[3], prev[4], prev[5])
                prev = (cA, atA, vzsA, cB, atB, vzsB)
            emit_o(prev[0], prev[1], prev[2])
            emit_o(prev[3], prev[4], prev[5])

        # ---------------- epilogue ----------------
        nc.scalar.activation(out=yt_sb[:, 0 : EU // 2],
                             in_=po[:, 0 : EU // 2], func=Copy)
        nc.vector.tensor_copy(yt_sb[:, EU // 2 : EU], po[:, EU // 2 : EU])
        for j, eng in enumerate((nc.sync, nc.gpsimd, nc.scalar, nc.sync)):
            eng.dma_start(out=yt_ap[:, 256 * j : 256 * (j + 1)],
                          in_=yt_sb[:, 256 * j : 256 * (j + 1)])


_CACHE = {}


def _build(gps_merge=True, warmup=10):
    key = ("nc", gps_merge, warmup)
    if key in _CACHE:
        return _CACHE[key]
    cfg = {"gps_merge": gps_merge, "warmup": warmup}
    nc = bacc.Bacc("TRN2", target_bir_lowering=False, debug=False,
                   enable_asserts=False, num_devices=1)
    xat = nc.dram_tensor("xat", (DA, L), F16, kind="ExternalInput").ap()
    w = nc.dram_tensor("wpack", (DA, 4 * D), F16, kind="ExternalInput").ap()
    yt = nc.dram_tensor("yt", (P, EU), F32, kind="ExternalOutput").ap()
    from contextlib import ExitStack
    with tile.TileContext(nc) as tc, ExitStack() as ctx:
        _attention_kernel(ctx, tc, yt, xat, w, cfg)
    nc.compile()
    _CACHE[key] = nc
    return nc


def _fold_weights(B_w, B_b, C_w, C_b, V_w, V_b):
    def aug(w, bias):
        full = np.concatenate(
            [np.asarray(w, np.float32).T, np.asarray(bias, np.float32)[None, :]],
            axis=0)
        return full.astype(np.float16)
    return aug(B_w, B_b), aug(C_w, C_b), aug(V_w, V_b)


def run(inputs, trace=False, tmpdir=None, gps_merge=True, warmup=10):
    nc = _build(gps_merge, warmup)
    x = np.asarray(inputs["x"], dtype=np.float32)
    b_augt, c_augt, v_augt = _fold_weights(
        inputs["B_w"], inputs["B_b"], inputs["C_w"], inputs["C_b"],
        inputs["V_w"], inputs["V_b"])
    ident = np.concatenate(
        [np.eye(D, dtype=np.float16), np.zeros((1, D), np.float16)], axis=0)
    wpack = np.concatenate([b_augt, c_augt, v_augt, ident], axis=1)
    ones = np.ones((1, L), np.float16)
    in_maps = []
    for i in range(N_CORES):
        xat = np.concatenate(
            [np.ascontiguousarray(x[i].T).astype(np.float16), ones], axis=0)
        in_maps.append({"xat": xat, "wpack": wpack})
    res = bass_utils.run_bass_kernel_spmd(nc, in_maps,
                                          core_ids=list(range(N_CORES)),
                                          trace=trace, tmpdir=tmpdir)
    out = np.empty((N_CORES, L, D), np.float32)
    for i in range(N_CORES):
        yt = res.results[i]["yt"]
        out[i, 0:EU, :] = yt[0:D, :].T
        out[i, EU : 2 * EU, :] = yt[D : 2 * D, :].T
    return out, res


def kernel(**inputs) -> np.ndarray:
    out, _ = run(inputs, trace=False)
    return out
